# revision 23
# baseline (speedup 1.0000x reference)
"""Trainium2 Bass kernel for nn_GCNStacking: 3-layer dense-adjacency GraphConv.

Per batch element b (one per NeuronCore, B=8 = n_cores=8, pure data parallel):
    H = relu(A @ (X @ Wm0^T) + X @ Ws0^T + b0)
    H = relu(A @ (H @ Wm1^T) + H @ Ws1^T + b1)
    H =      A @ (H @ Wm2^T) + H @ Ws2^T + b2

Dataflow (per core), state kept transposed, Ht = H^T [C=64, N=2048]:
  - A^T is materialized once in SBUF via pipelined PE transposes while the
    row-slabs of A stream in from HBM (quarter-width pieces, descriptors
    sprayed round-robin over the 16 DMA queues); reused by all 3 layers.
  - Message M_l (natural [N, C]) via 16 small fp16 matmuls (1 cyc/row;
    f32r pays a 4x small-moving penalty at ap=64).
  - Aggregation Ot = (A@M)^T: lhsT = M-block [128,64] stationary,
    rhs = A^T-block [128,512] moving, accumulated over 16 j-blocks + self
    term into one PSUM bank; layer-1 runs software-pipelined one chunk
    behind the transposes.
  - bias+relu evacuation on Scalar writes Ht fp16 directly; final layer
    evacuates per-128-column piece (f32: outputs reach ~1.3e7, beyond fp16
    range) -> PE transpose -> copy -> DMA, draining the tail incrementally.
  - X is loaded flat ([128, 1024], 128 descriptors instead of 2048) and
    Ht0 = X^T is assembled from PE transposes of the flat tile via
    stride-16 column writes on DVE/ACT.

GCN_AGG_MODE selects how A reaches the PE transposes:
  "f16dma" (default): gpsimd-initiated casting DMA loads A f32->fp16 in the
      DMA itself; transposes and all matmuls run at 16-bit rate (1 cyc/row).
  "f16op": sync DMA loads f32; idle GpSimd casts to fp16; same PE path.
  "f32r": f32 pieces, f32 transposes, f32r aggregation operands (legacy).
"""
import sys

for _p in ("/opt/trn_rl_repo",):
    if _p not in sys.path:
        sys.path.insert(0, _p)

import numpy as np
import orjson

import concourse.bass as bass
import concourse.tile as tile
from concourse import mybir
from concourse.bass import _add_dep_helper as add_dep

f32 = mybir.dt.float32
f32r = mybir.dt.float32r
bf16 = mybir.dt.bfloat16
f16 = mybir.dt.float16

import os as _os
AGG_MODE = _os.environ.get("GCN_AGG_MODE", "f16dma")

# ---------------------------------------------------------------------------
# Workaround: this walrus build accepts at most ONE embedded sync-wait per
# instruction ("Too many sync wait commands").  Split excess waits onto
# inserted NoOps (same engine, right before the host instruction).
# ---------------------------------------------------------------------------
_ws_ctr = [0]


def _split_waits_json(bir_bytes: bytes) -> bytes:
    d = orjson.loads(bir_bytes)
    changed = False
    for fn in d.get("functions", []):
        for blk in fn.get("blocks", []):
            out = []
            for inst in blk.get("instructions", []):
                si = inst.get("sync_info")
                waits = (si or {}).get("on_wait") or []
                eng = inst.get("engine")
                if len(waits) > 1 and eng and eng != "Unassigned":
                    changed = True
                    for w in waits[:-1]:
                        _ws_ctr[0] += 1
                        out.append({
                            "name": f"I-wsplit-{_ws_ctr[0]}",
                            "opcode": "NoOp",
                            "engine": eng,
                            "ins": [],
                            "outs": [],
                            "sync_info": {"on_wait": [w], "on_update": []},
                        })
                    si["on_wait"] = waits[-1:]
                out.append(inst)
            blk["instructions"] = out
    return orjson.dumps(d) if changed else bir_bytes


def _install_waitsplit():
    from concourse import bass2jax, bass_utils
    if getattr(bass_utils, "_waitsplit_installed", False):
        return
    orig = bass_utils.compile_bir_kernel

    def patched(bir_json, tmpdir, neff_name="file.neff"):
        return orig(_split_waits_json(bytes(bir_json)), tmpdir, neff_name=neff_name)

    bass_utils.compile_bir_kernel = patched
    bass2jax.compile_bir_kernel = patched
    bass_utils._waitsplit_installed = True


_install_waitsplit()

# ---------------------------------------------------------------------------
# Kernel builder
# ---------------------------------------------------------------------------
P = 128
C = 64
N_LAYERS = 3


def build_gcn(nn_nodes: int = 2048):
    """Build the single-core Bass program; the same program runs SPMD on all
    8 cores with per-core (per-batch) inputs."""
    NN = nn_nodes
    NB = NN // P            # node blocks (16)
    CH = 512                # aggregation i-chunk (one PSUM bank of f32)
    IC = NN // CH           # i-chunks (4)
    SLAB_PAIRS = 2          # A-slabs transposed per psum tile
    QNN = NN // 4           # quarter-width A pieces (2KB rows)
    NQ = 4

    f16_mode = AGG_MODE in ("f16dma", "f16op")
    cast_in_dma = AGG_MODE == "f16dma"
    # adt: aggregation operand dtype (ATr, mn) = A-transpose dtype
    adt = f16 if f16_mode else f32r
    # pdt: dtype of the A pieces as fed to the PE transposes
    pdt = f16 if f16_mode else f32
    # sdt: layer state (Ht, wT): fp16 makes the ap=64 message matmuls
    # 1 cyc/row instead of f32r's 4
    sdt = f16
    tdt = f32               # X/W/b transpose-source dtype

    nc = bass.Bass()
    X_in = nc.declare_dram_parameter("X", [NN, C], tdt, isOutput=False)
    A_in = nc.declare_dram_parameter("A", [NN, NN], tdt, isOutput=False)
    W_in = {}
    b_in = {}
    for l in range(N_LAYERS):
        W_in[(l, "m")] = nc.declare_dram_parameter(f"Wm{l}", [C, C], tdt, isOutput=False)
        W_in[(l, "s")] = nc.declare_dram_parameter(f"Ws{l}", [C, C], tdt, isOutput=False)
        b_in[l] = nc.declare_dram_parameter(f"b{l}", [C], tdt, isOutput=False)
    H_out = nc.declare_dram_parameter("H", [NN, C], f32, isOutput=True)

    with tile.TileContext(nc) as tc:
        with (
            tc.tile_pool(name="const", bufs=1) as const,
            tc.tile_pool(name="ht_pool", bufs=2) as ht_pool,
            tc.tile_pool(name="mn_pool", bufs=2) as mn_pool,
            tc.tile_pool(name="slab_pool", bufs=20) as slab_pool,
            tc.tile_pool(name="u_pool", bufs=4) as u_pool,
            tc.tile_pool(name="hb_pool", bufs=4) as hb_pool,
            tc.tile_pool(name="ps_tr", bufs=3, space="PSUM") as ps_tr,
            tc.tile_pool(name="ps_o", bufs=2, space="PSUM") as ps_o,
            tc.tile_pool(name="ps_m", bufs=1, space="PSUM") as ps_m,
        ):
            # ---- phase 0: constants on DVE/GpSimd, input DMAs -------------
            junk = const.tile([P, C], f32, name="junk")
            jk_i = nc.vector.memset(junk, 0.0)
            ident = const.tile([P, P], tdt, name="ident")
            id_i1 = nc.vector.memset(ident, 0.0)
            id_i2 = nc.gpsimd.affine_select(
                out=ident, in_=ident,
                compare_op=mybir.AluOpType.not_equal,
                fill=1.0, base=0, pattern=[[-1, P]], channel_multiplier=1,
            )
            id_deps = [id_i1, id_i2]
            if f16_mode:
                ident16 = const.tile([P, P], f16, name="ident16")
                id16_i1 = nc.vector.memset(ident16, 0.0)
                id16_i2 = nc.gpsimd.affine_select(
                    out=ident16, in_=ident16,
                    compare_op=mybir.AluOpType.not_equal,
                    fill=1.0, base=0, pattern=[[-1, P]], channel_multiplier=1,
                )
                id_deps += [id16_i1, id16_i2]

            # X flat: partition p holds rows 16p..16p+15 (one 4KB descriptor
            # per partition instead of 16 x 256B)
            XR = NN * C // P    # 1024
            x_flat = const.tile([P, XR], tdt, name="x_flat")
            x_dma = nc.sync.dma_start(
                x_flat, X_in[:].rearrange("(p f) c -> p (f c)", p=P))

            b_row = {}
            for l in range(N_LAYERS):
                br = const.tile([1, C], tdt, name=f"b_row{l}")
                nc.sync.dma_start(br, b_in[l][:].rearrange("(o c) -> o c", o=1))
                b_row[l] = br

            w_stage = {}
            w_dmas = []
            for l in range(N_LAYERS):
                for kind in ("m", "s"):
                    wst = const.tile([C, C], tdt, name=f"wst_{l}{kind}")
                    w_dmas.append(nc.sync.dma_start(wst, W_in[(l, kind)][:]))
                    w_stage[(l, kind)] = wst

            # ---- A prefetch right after the small input DMAs --------------
            prefetch = {}

            def a_piece(pf_key, s, q):
                """Load A piece (rows s*P.., cols q*QNN..) and return the
                fp16/f32 tile the PE transposes consume + its ready dep."""
                src = A_in[s * P:(s + 1) * P, q * QNN:(q + 1) * QNN]
                if cast_in_dma:
                    a_pc = slab_pool.tile([P, QNN], f16, name="a_pc",
                                          tag="aslab")
                    d = nc.gpsimd.dma_start(a_pc, src)
                else:
                    a_pc = slab_pool.tile([P, QNN], f32, name="a_pc",
                                          tag="aslab")
                    d = nc.sync.dma_start(a_pc, src)
                    if f16_mode:
                        a16 = slab_pool.tile([P, QNN], f16, name="a16",
                                             tag="a16slab")
                        d = nc.gpsimd.tensor_copy(a16, a_pc)
                        a_pc = a16
                prefetch[pf_key] = (a_pc, d)

            for pf_pair in range(2):
                for q in range(NQ):
                    for si in range(SLAB_PAIRS):
                        a_piece((pf_pair, si, q), pf_pair * SLAB_PAIRS + si, q)

            # gate: one PE nop absorbing phase-0 input waits so the
            # transposes below carry at most one embedded wait each
            gate0 = nc.tensor.nop(nofuse=True)
            for d in (*id_deps, x_dma, *w_dmas):
                add_dep(gate0.ins, d.ins, True, "phase0 gate")

            # warm-up matmuls on the junk tile: engage the PE HAM clock-gate
            # (~3.4us of sustained activity -> 2.4 GHz) before real data
            # arrives, so the first A transposes run at full clock
            warm_gate = nc.tensor.nop(nofuse=True)
            add_dep(warm_gate.ins, jk_i.ins, True, "warmup gate")
            pwarm = ps_m.tile([P, C], f32, name="pwarm", tag="m")
            for wi in range(28):
                wmm = nc.tensor.matmul(pwarm[:C, :], junk, junk,
                                       start=True, stop=True,
                                       skip_group_check=True)
                if wi == 0:
                    add_dep(wmm.ins, warm_gate.ins, False, "after warmup gate")

            # Ht[l]: transposed state [C, NN]; Ht[0] = X^T
            Ht = [ht_pool.tile([C, NN], sdt, name=f"Ht{l}", tag="ht")
                  for l in range(N_LAYERS)]

            # X^T assembly: transpose flat tile; piece k yields columns
            # n = 16p + 2k + r (r in {0,1}) -> stride-16 writes into Ht0.
            ht0_v = Ht[0][:].rearrange("c (p m) -> c m p", m=NB)
            for k in range(XR // P):
                pt = ps_tr.tile([P, P], tdt, name="pt_x", tag="trx", bufs=2)
                t = nc.tensor.transpose(pt, x_flat[:, k * P:(k + 1) * P],
                                        ident)
                add_dep(t.ins, gate0.ins, False, "after gate0")
                nc.vector.tensor_copy(ht0_v[:, 2 * k, :], pt[:C, :])
                nc.scalar.copy(ht0_v[:, 2 * k + 1, :], pt[C:, :])

            wT = {}
            for (l, kind), wst in w_stage.items():
                pw = ps_tr.tile([P, P], tdt, name="pt_w", tag="trx", bufs=2)
                t = nc.tensor.transpose(pw[:C, :C], wst, ident[:C, :C])
                add_dep(t.ins, gate0.ins, False, "after gate0")
                wt = const.tile([C, C], sdt, name=f"wT_{l}{kind}")
                nc.vector.tensor_copy(wt, pw[:C, :C])
                wT[(l, kind)] = wt

            b_sb = {}
            for l in range(N_LAYERS):
                pb = ps_tr.tile([P, 1], tdt, name="pt_b", tag="trx", bufs=2)
                nc.tensor.transpose(pb[:C, :], b_row[l], ident[:1, :1])
                bt = const.tile([C, 1], f32, name=f"b_sb{l}")
                nc.scalar.copy(bt, pb[:C, :])
                b_sb[l] = bt

            # resident A^T [j-partition, j-block, i]
            ATr = const.tile([P, NB, NN], adt, name="ATr")

            def emit_mprod(l, mn, jbs=None):
                """M_l natural [N, C] blocks: lhsT = Ht[l] block, rhs = WmT."""
                for jb in (range(NB) if jbs is None else jbs):
                    pm = ps_m.tile([P, C], f32, name="pm", tag="m")
                    nc.tensor.matmul(pm, Ht[l][:, jb * P:(jb + 1) * P],
                                     wT[(l, "m")], start=True, stop=True)
                    # Mn copies on Scalar (ACT) to keep DVE free
                    nc.scalar.copy(mn[:, jb, :], pm)

            def emit_evac(l, g, po):
                if l < N_LAYERS - 1:
                    nc.scalar.activation(
                        Ht[l + 1][:, g * CH:(g + 1) * CH], po[:C, :],
                        mybir.ActivationFunctionType.Relu,
                        bias=b_sb[l], scale=1.0)
                    return
                # final layer: per-128-piece evac -> transpose -> DMA, so
                # the tail drains incrementally.  f32 throughout: the final
                # outputs reach ~1.3e7, far beyond fp16 range.
                for k in range(CH // P):
                    ho = u_pool.tile([C, P], f32, name="ho", tag="ho")
                    nc.scalar.activation(ho, po[:C, k * P:(k + 1) * P],
                                         mybir.ActivationFunctionType.Identity,
                                         bias=b_sb[l], scale=1.0)
                    ph = ps_tr.tile([P, C], f32, name="ph", tag="trx", bufs=2)
                    nc.tensor.transpose(ph, ho, ident[:C, :C])
                    hb = hb_pool.tile([P, C], f32, name="hb", tag="hb")
                    nc.vector.tensor_copy(hb, ph)
                    r0 = g * CH + k * P
                    nc.sync.dma_start(H_out[r0:r0 + P, :], hb)

            # ---- layer 1, pipelined with the A load/transpose -------------
            mn1 = mn_pool.tile([P, NB, C], adt, name="mn", tag="mn")
            emit_mprod(0, mn1)

            # bridge dummies: keep the PE HAM-warm across the short wait for
            # the first A pieces (a re-throttle here costs ~25us of half-clock
            # transposes in unlucky runs)
            for _bi in range(14):
                nc.tensor.matmul(pwarm[:C, :], junk, junk,
                                 start=True, stop=True, skip_group_check=True)

            def cast_copy(eng_idx, dst, srcp):
                # 3:1 DVE:ACT — ACT copies are ~1.6x slower and ACT also
                # carries the Mn copies and evacuations
                if eng_idx % 4 != 3:
                    nc.vector.tensor_copy(dst, srcp)
                else:
                    nc.scalar.copy(dst, srcp)

            tr_ident = ident16 if f16_mode else ident

            def emit_transpose_pair(g, pair):
                """Transpose 2 row-slabs of A (each loaded as 4 quarter-width
                pieces) into ATr columns, chunk g."""
                s0 = g * (CH // P) + pair * SLAB_PAIRS
                pieces = {}
                if g == 0 and pair < 2:
                    for q in range(NQ):
                        for si in range(SLAB_PAIRS):
                            pieces[(si, q)] = prefetch[(pair, si, q)]
                else:
                    for q in range(NQ):
                        for si in range(SLAB_PAIRS):
                            k = (g, pair, si, q)
                            a_piece(k, s0 + si, q)
                            pieces[(si, q)] = prefetch.pop(k)
                c0 = g * CH + pair * SLAB_PAIRS * P
                for q in range(NQ):
                    gate = nc.tensor.nop(nofuse=True)
                    for si in range(SLAB_PAIRS):
                        add_dep(gate.ins, pieces[(si, q)][1].ins, True,
                                "piece gate")
                    srcs = [pieces[(si, q)][0] for si in range(SLAB_PAIRS)]
                    for jbl in range(QNN // P):
                        jb = q * (QNN // P) + jbl
                        pt = ps_tr.tile([P, SLAB_PAIRS * P], pdt, name="pt_a",
                                        tag="tr")
                        for si in range(SLAB_PAIRS):
                            t = nc.tensor.transpose(
                                pt[:, si * P:(si + 1) * P],
                                srcs[si][:, jbl * P:(jbl + 1) * P],
                                tr_ident)
                            add_dep(t.ins, gate.ins, False, "after piece gate")
                        cast_copy(jb, ATr[:, jb, c0:c0 + SLAB_PAIRS * P], pt)

            # agg chunk split into two emission halves for interleaving
            open_po = {}

            def emit_agg_half(l, g, mn, half):
                cs = slice(g * CH, (g + 1) * CH)
                if half == 0:
                    po = ps_o.tile([C, CH], f32, name="po", tag="o")
                    open_po[(l, g)] = po
                    for jb in range(NB // 2):
                        nc.tensor.matmul(
                            po, mn[:, jb, :], ATr[:, jb, cs],
                            start=(jb == 0), stop=False,
                            skip_group_check=True)
                else:
                    po = open_po.pop((l, g))
                    for jb in range(NB // 2, NB):
                        nc.tensor.matmul(
                            po, mn[:, jb, :], ATr[:, jb, cs],
                            start=False, stop=False,
                            skip_group_check=True)
                    nc.tensor.matmul(
                        po, wT[(l, "s")], Ht[l][:, cs],
                        start=False, stop=True, skip_group_check=True)
                    emit_evac(l, g, po)

            for g in range(IC):
                for pair in range(CH // (P * SLAB_PAIRS)):
                    emit_transpose_pair(g, pair)
                    if g > 0:
                        emit_agg_half(0, g - 1, mn1, pair)
            emit_agg_half(0, IC - 1, mn1, 0)
            emit_agg_half(0, IC - 1, mn1, 1)

            # ---- layers 2..3 ---------------------------------------------
            # Mprod for layer l+1 is interleaved right after layer l's chunk
            # g evacuates (its Ht columns are ready), smoothing the PE stream
            # across layer boundaries.
            mns = {0: mn1}
            for l in range(1, N_LAYERS):
                mns[l] = mn_pool.tile([P, NB, C], adt, name="mn", tag="mn")
            for l in range(1, N_LAYERS):
                mn = mns[l]
                if l == 1:
                    emit_mprod(l, mn)  # Ht[1] fully available by now
                for g in range(IC):
                    emit_agg_half(l, g, mn, 0)
                    emit_agg_half(l, g, mn, 1)
                    if l + 1 < N_LAYERS:
                        jb0 = g * (NB // IC)
                        emit_mprod(l + 1, mns[l + 1],
                                   range(jb0, jb0 + NB // IC))

    return nc


# ---------------------------------------------------------------------------
# Harness entry point
# ---------------------------------------------------------------------------
_NC_CACHE = {}


def _get_nc(nn_nodes):
    if nn_nodes not in _NC_CACHE:
        _NC_CACHE[nn_nodes] = build_gcn(nn_nodes)
    return _NC_CACHE[nn_nodes]


def kernel(X, A, Wm0, Ws0, b0, Wm1, Ws1, b1, Wm2, Ws2, b2, _trace=False):
    from concourse.bass_utils import run_bass_kernel_spmd

    X = np.ascontiguousarray(np.asarray(X, dtype=np.float32))
    A = np.ascontiguousarray(np.asarray(A, dtype=np.float32))
    B, NN, _C = X.shape
    assert B == 8, f"expected batch 8 (one per core), got {B}"

    shared = {
        "Wm0": np.ascontiguousarray(np.asarray(Wm0, np.float32)),
        "Ws0": np.ascontiguousarray(np.asarray(Ws0, np.float32)),
        "b0": np.ascontiguousarray(np.asarray(b0, np.float32)),
        "Wm1": np.ascontiguousarray(np.asarray(Wm1, np.float32)),
        "Ws1": np.ascontiguousarray(np.asarray(Ws1, np.float32)),
        "b1": np.ascontiguousarray(np.asarray(b1, np.float32)),
        "Wm2": np.ascontiguousarray(np.asarray(Wm2, np.float32)),
        "Ws2": np.ascontiguousarray(np.asarray(Ws2, np.float32)),
        "b2": np.ascontiguousarray(np.asarray(b2, np.float32)),
    }
    nc = _get_nc(NN)
    in_maps = [dict(shared, X=X[b], A=A[b]) for b in range(B)]
    res = run_bass_kernel_spmd(nc, in_maps, core_ids=list(range(B)),
                               trace=_trace)
    out = np.stack([res.results[b]["H"] for b in range(B)], axis=0)
    if _trace:
        return out, res
    return out


# revision 24
# speedup vs baseline: 1.3457x; 1.3457x over previous
"""Trainium2 Bass kernel for nn_GCNStacking: 3-layer dense-adjacency GraphConv.

Per batch element b (one per NeuronCore, B=8 = n_cores=8, pure data parallel):
    H = relu(A @ (X @ Wm0^T) + X @ Ws0^T + b0)
    H = relu(A @ (H @ Wm1^T) + H @ Ws1^T + b1)
    H =      A @ (H @ Wm2^T) + H @ Ws2^T + b2

Dataflow (per core), state kept transposed, Ht = H^T [C=64, N=2048]:
  - A^T is materialized once in SBUF via pipelined PE transposes while the
    row-slabs of A stream in from HBM (quarter-width pieces, descriptors
    sprayed round-robin over the 16 DMA queues); reused by all 3 layers.
  - Message M_l (natural [N, C]) via 16 small fp16 matmuls (1 cyc/row;
    f32r pays a 4x small-moving penalty at ap=64).
  - Aggregation Ot = (A@M)^T: lhsT = M-block [128,64] stationary,
    rhs = A^T-block [128,512] moving, accumulated over 16 j-blocks + self
    term into one PSUM bank; layer-1 runs software-pipelined one chunk
    behind the transposes.
  - bias+relu evacuation on Scalar writes Ht fp16 directly; final layer
    evacuates per-128-column piece (f32: outputs reach ~1.3e7, beyond fp16
    range) -> PE transpose -> copy -> DMA, draining the tail incrementally.
  - X is loaded flat ([128, 1024], 128 descriptors instead of 2048) and
    Ht0 = X^T is assembled from PE transposes of the flat tile via
    stride-16 column writes on DVE/ACT.

GCN_AGG_MODE selects how A reaches the PE transposes:
  "f16dma" (default): gpsimd-initiated casting DMA loads A f32->fp16 in the
      DMA itself; transposes and all matmuls run at 16-bit rate (1 cyc/row).
  "f16op": sync DMA loads f32; idle GpSimd casts to fp16; same PE path.
  "f32r": f32 pieces, f32 transposes, f32r aggregation operands (legacy).
"""
import sys

for _p in ("/opt/trn_rl_repo",):
    if _p not in sys.path:
        sys.path.insert(0, _p)

import numpy as np
import orjson

import concourse.bass as bass
import concourse.tile as tile
from concourse import mybir
from concourse.bass import _add_dep_helper as add_dep

f32 = mybir.dt.float32
f32r = mybir.dt.float32r
bf16 = mybir.dt.bfloat16
f16 = mybir.dt.float16

import os as _os
AGG_MODE = _os.environ.get("GCN_AGG_MODE", "f16dma")

# ---------------------------------------------------------------------------
# Workaround: this walrus build accepts at most ONE embedded sync-wait per
# instruction ("Too many sync wait commands").  Split excess waits onto
# inserted NoOps (same engine, right before the host instruction).
# ---------------------------------------------------------------------------
_ws_ctr = [0]


def _split_waits_json(bir_bytes: bytes) -> bytes:
    d = orjson.loads(bir_bytes)
    changed = False
    for fn in d.get("functions", []):
        for blk in fn.get("blocks", []):
            out = []
            for inst in blk.get("instructions", []):
                si = inst.get("sync_info")
                waits = (si or {}).get("on_wait") or []
                eng = inst.get("engine")
                if len(waits) > 1 and eng and eng != "Unassigned":
                    changed = True
                    for w in waits[:-1]:
                        _ws_ctr[0] += 1
                        out.append({
                            "name": f"I-wsplit-{_ws_ctr[0]}",
                            "opcode": "NoOp",
                            "engine": eng,
                            "ins": [],
                            "outs": [],
                            "sync_info": {"on_wait": [w], "on_update": []},
                        })
                    si["on_wait"] = waits[-1:]
                out.append(inst)
            blk["instructions"] = out
    return orjson.dumps(d) if changed else bir_bytes


def _install_waitsplit():
    from concourse import bass2jax, bass_utils
    if getattr(bass_utils, "_waitsplit_installed", False):
        return
    orig = bass_utils.compile_bir_kernel

    def patched(bir_json, tmpdir, neff_name="file.neff"):
        return orig(_split_waits_json(bytes(bir_json)), tmpdir, neff_name=neff_name)

    bass_utils.compile_bir_kernel = patched
    bass2jax.compile_bir_kernel = patched
    bass_utils._waitsplit_installed = True


_install_waitsplit()

# ---------------------------------------------------------------------------
# Kernel builder
# ---------------------------------------------------------------------------
P = 128
C = 64
N_LAYERS = 3


def build_gcn(nn_nodes: int = 2048):
    """Build the single-core Bass program; the same program runs SPMD on all
    8 cores with per-core (per-batch) inputs."""
    NN = nn_nodes
    NB = NN // P            # node blocks (16)
    CH = 512                # aggregation i-chunk (one PSUM bank of f32)
    IC = NN // CH           # i-chunks (4)
    SLAB_PAIRS = 2          # A-slabs transposed per psum tile
    QNN = NN // 4           # quarter-width A pieces (2KB rows)
    NQ = 4

    f16_mode = AGG_MODE in ("f16dma", "f16op")
    cast_in_dma = AGG_MODE == "f16dma"
    # adt: aggregation operand dtype (ATr, mn) = A-transpose dtype
    adt = f16 if f16_mode else f32r
    # pdt: dtype of the A pieces as fed to the PE transposes
    pdt = f16 if f16_mode else f32
    # sdt: layer state (Ht, wT): fp16 makes the ap=64 message matmuls
    # 1 cyc/row instead of f32r's 4
    sdt = f16
    tdt = f32               # X/W/b transpose-source dtype

    nc = bass.Bass()
    X_in = nc.declare_dram_parameter("X", [NN, C], tdt, isOutput=False)
    A_in = nc.declare_dram_parameter("A", [NN, NN], tdt, isOutput=False)
    W_in = {}
    b_in = {}
    for l in range(N_LAYERS):
        W_in[(l, "m")] = nc.declare_dram_parameter(f"Wm{l}", [C, C], tdt, isOutput=False)
        W_in[(l, "s")] = nc.declare_dram_parameter(f"Ws{l}", [C, C], tdt, isOutput=False)
        b_in[l] = nc.declare_dram_parameter(f"b{l}", [C], tdt, isOutput=False)
    H_out = nc.declare_dram_parameter("H", [NN, C], f32, isOutput=True)

    with tile.TileContext(nc) as tc:
        with (
            tc.tile_pool(name="const", bufs=1) as const,
            tc.tile_pool(name="ht_pool", bufs=2) as ht_pool,
            tc.tile_pool(name="mn_pool", bufs=2) as mn_pool,
            tc.tile_pool(name="slab_pool",
                         bufs=32 if AGG_MODE == "f16dma" else 20) as slab_pool,
            tc.tile_pool(name="u_pool", bufs=4) as u_pool,
            tc.tile_pool(name="hb_pool", bufs=4) as hb_pool,
            tc.tile_pool(name="ps_tr", bufs=3, space="PSUM") as ps_tr,
            tc.tile_pool(name="ps_o", bufs=2, space="PSUM") as ps_o,
            tc.tile_pool(name="ps_m", bufs=1, space="PSUM") as ps_m,
        ):
            # ---- phase 0: constants on DVE/GpSimd, input DMAs -------------
            junk = const.tile([P, C], f32, name="junk")
            jk_i = nc.vector.memset(junk, 0.0)
            ident = const.tile([P, P], tdt, name="ident")
            id_i1 = nc.vector.memset(ident, 0.0)
            id_i2 = nc.gpsimd.affine_select(
                out=ident, in_=ident,
                compare_op=mybir.AluOpType.not_equal,
                fill=1.0, base=0, pattern=[[-1, P]], channel_multiplier=1,
            )
            id_deps = [id_i1, id_i2]
            if f16_mode:
                ident16 = const.tile([P, P], f16, name="ident16")
                id16_i1 = nc.vector.memset(ident16, 0.0)
                id16_i2 = nc.gpsimd.affine_select(
                    out=ident16, in_=ident16,
                    compare_op=mybir.AluOpType.not_equal,
                    fill=1.0, base=0, pattern=[[-1, P]], channel_multiplier=1,
                )
                id_deps += [id16_i1, id16_i2]

            # X flat: partition p holds rows 16p..16p+15 (one 4KB descriptor
            # per partition instead of 16 x 256B)
            XR = NN * C // P    # 1024
            x_flat = const.tile([P, XR], tdt, name="x_flat")
            x_dma = nc.sync.dma_start(
                x_flat, X_in[:].rearrange("(p f) c -> p (f c)", p=P))

            b_row = {}
            for l in range(N_LAYERS):
                br = const.tile([1, C], tdt, name=f"b_row{l}")
                nc.sync.dma_start(br, b_in[l][:].rearrange("(o c) -> o c", o=1))
                b_row[l] = br

            w_stage = {}
            w_dmas = []
            for l in range(N_LAYERS):
                for kind in ("m", "s"):
                    wst = const.tile([C, C], tdt, name=f"wst_{l}{kind}")
                    w_dmas.append(nc.sync.dma_start(wst, W_in[(l, kind)][:]))
                    w_stage[(l, kind)] = wst

            # ---- A prefetch right after the small input DMAs --------------
            prefetch = {}

            def a_piece(pf_key, s, q):
                """Load A piece (rows s*P.., cols q*QNN..) and return the
                fp16/f32 tile the PE transposes consume + its ready dep."""
                src = A_in[s * P:(s + 1) * P, q * QNN:(q + 1) * QNN]
                if cast_in_dma:
                    a_pc = slab_pool.tile([P, QNN], f16, name="a_pc",
                                          tag="aslab")
                    d = nc.gpsimd.dma_start(a_pc, src)
                else:
                    a_pc = slab_pool.tile([P, QNN], f32, name="a_pc",
                                          tag="aslab")
                    d = nc.sync.dma_start(a_pc, src)
                    if f16_mode:
                        a16 = slab_pool.tile([P, QNN], f16, name="a16",
                                             tag="a16slab")
                        d = nc.gpsimd.tensor_copy(a16, a_pc)
                        a_pc = a16
                prefetch[pf_key] = (a_pc, d)

            for pf_pair in range(2):
                for q in range(NQ):
                    for si in range(SLAB_PAIRS):
                        a_piece((pf_pair, si, q), pf_pair * SLAB_PAIRS + si, q)

            # gate: one PE nop absorbing phase-0 input waits so the
            # transposes below carry at most one embedded wait each
            gate0 = nc.tensor.nop(nofuse=True)
            for d in (*id_deps, x_dma, *w_dmas):
                add_dep(gate0.ins, d.ins, True, "phase0 gate")

            # warm-up matmuls on the junk tile: engage the PE HAM clock-gate
            # (~3.4us of sustained activity -> 2.4 GHz) before real data
            # arrives, so the first A transposes run at full clock
            warm_gate = nc.tensor.nop(nofuse=True)
            add_dep(warm_gate.ins, jk_i.ins, True, "warmup gate")
            pwarm = ps_m.tile([P, C], f32, name="pwarm", tag="m")
            for wi in range(28):
                wmm = nc.tensor.matmul(pwarm[:C, :], junk, junk,
                                       start=True, stop=True,
                                       skip_group_check=True)
                if wi == 0:
                    add_dep(wmm.ins, warm_gate.ins, False, "after warmup gate")

            # Ht[l]: transposed state [C, NN]; Ht[0] = X^T
            Ht = [ht_pool.tile([C, NN], sdt, name=f"Ht{l}", tag="ht")
                  for l in range(N_LAYERS)]

            # X^T assembly: transpose flat tile; piece k yields columns
            # n = 16p + 2k + r (r in {0,1}) -> stride-16 writes into Ht0.
            ht0_v = Ht[0][:].rearrange("c (p m) -> c m p", m=NB)
            for k in range(XR // P):
                pt = ps_tr.tile([P, P], tdt, name="pt_x", tag="trx", bufs=2)
                t = nc.tensor.transpose(pt, x_flat[:, k * P:(k + 1) * P],
                                        ident)
                add_dep(t.ins, gate0.ins, False, "after gate0")
                nc.vector.tensor_copy(ht0_v[:, 2 * k, :], pt[:C, :])
                nc.scalar.copy(ht0_v[:, 2 * k + 1, :], pt[C:, :])

            wT = {}
            for (l, kind), wst in w_stage.items():
                pw = ps_tr.tile([P, P], tdt, name="pt_w", tag="trx", bufs=2)
                t = nc.tensor.transpose(pw[:C, :C], wst, ident[:C, :C])
                add_dep(t.ins, gate0.ins, False, "after gate0")
                wt = const.tile([C, C], sdt, name=f"wT_{l}{kind}")
                nc.vector.tensor_copy(wt, pw[:C, :C])
                wT[(l, kind)] = wt

            b_sb = {}
            for l in range(N_LAYERS):
                pb = ps_tr.tile([P, 1], tdt, name="pt_b", tag="trx", bufs=2)
                nc.tensor.transpose(pb[:C, :], b_row[l], ident[:1, :1])
                bt = const.tile([C, 1], f32, name=f"b_sb{l}")
                nc.scalar.copy(bt, pb[:C, :])
                b_sb[l] = bt

            # resident A^T [j-partition, j-block, i]
            ATr = const.tile([P, NB, NN], adt, name="ATr")

            def emit_mprod(l, mn, jbs=None):
                """M_l natural [N, C] blocks: lhsT = Ht[l] block, rhs = WmT."""
                for jb in (range(NB) if jbs is None else jbs):
                    pm = ps_m.tile([P, C], f32, name="pm", tag="m")
                    nc.tensor.matmul(pm, Ht[l][:, jb * P:(jb + 1) * P],
                                     wT[(l, "m")], start=True, stop=True)
                    # Mn copies on Scalar (ACT) to keep DVE free
                    nc.scalar.copy(mn[:, jb, :], pm)

            def emit_evac(l, g, po):
                if l < N_LAYERS - 1:
                    nc.scalar.activation(
                        Ht[l + 1][:, g * CH:(g + 1) * CH], po[:C, :],
                        mybir.ActivationFunctionType.Relu,
                        bias=b_sb[l], scale=1.0)
                    return
                # final layer: per-128-piece evac -> transpose -> DMA, so
                # the tail drains incrementally.  f32 throughout: the final
                # outputs reach ~1.3e7, far beyond fp16 range.
                for k in range(CH // P):
                    ho = u_pool.tile([C, P], f32, name="ho", tag="ho")
                    nc.scalar.activation(ho, po[:C, k * P:(k + 1) * P],
                                         mybir.ActivationFunctionType.Identity,
                                         bias=b_sb[l], scale=1.0)
                    ph = ps_tr.tile([P, C], f32, name="ph", tag="trx", bufs=2)
                    nc.tensor.transpose(ph, ho, ident[:C, :C])
                    hb = hb_pool.tile([P, C], f32, name="hb", tag="hb")
                    nc.vector.tensor_copy(hb, ph)
                    r0 = g * CH + k * P
                    nc.sync.dma_start(H_out[r0:r0 + P, :], hb)

            # ---- layer 1, pipelined with the A load/transpose -------------
            mn1 = mn_pool.tile([P, NB, C], adt, name="mn", tag="mn")
            emit_mprod(0, mn1)

            # bridge dummies: keep the PE HAM-warm across the short wait for
            # the first A pieces (a re-throttle here costs ~25us of half-clock
            # transposes in unlucky runs)
            for _bi in range(14):
                nc.tensor.matmul(pwarm[:C, :], junk, junk,
                                 start=True, stop=True, skip_group_check=True)

            def cast_copy(eng_idx, dst, srcp):
                # 3:1 DVE:ACT — ACT copies are ~1.6x slower and ACT also
                # carries the Mn copies and evacuations
                if eng_idx % 4 != 3:
                    nc.vector.tensor_copy(dst, srcp)
                else:
                    nc.scalar.copy(dst, srcp)

            tr_ident = ident16 if f16_mode else ident

            def emit_transpose_pair(g, pair):
                """Transpose 2 row-slabs of A (each loaded as 4 quarter-width
                pieces) into ATr columns, chunk g."""
                s0 = g * (CH // P) + pair * SLAB_PAIRS
                pieces = {}
                if g == 0 and pair < 2:
                    for q in range(NQ):
                        for si in range(SLAB_PAIRS):
                            pieces[(si, q)] = prefetch[(pair, si, q)]
                else:
                    for q in range(NQ):
                        for si in range(SLAB_PAIRS):
                            k = (g, pair, si, q)
                            a_piece(k, s0 + si, q)
                            pieces[(si, q)] = prefetch.pop(k)
                c0 = g * CH + pair * SLAB_PAIRS * P
                for q in range(NQ):
                    gate = nc.tensor.nop(nofuse=True)
                    for si in range(SLAB_PAIRS):
                        add_dep(gate.ins, pieces[(si, q)][1].ins, True,
                                "piece gate")
                    srcs = [pieces[(si, q)][0] for si in range(SLAB_PAIRS)]
                    for jbl in range(QNN // P):
                        jb = q * (QNN // P) + jbl
                        pt = ps_tr.tile([P, SLAB_PAIRS * P], pdt, name="pt_a",
                                        tag="tr")
                        for si in range(SLAB_PAIRS):
                            t = nc.tensor.transpose(
                                pt[:, si * P:(si + 1) * P],
                                srcs[si][:, jbl * P:(jbl + 1) * P],
                                tr_ident)
                            add_dep(t.ins, gate.ins, False, "after piece gate")
                        cast_copy(jb, ATr[:, jb, c0:c0 + SLAB_PAIRS * P], pt)

            # agg chunk split into two emission halves for interleaving
            open_po = {}

            def emit_agg_half(l, g, mn, half):
                cs = slice(g * CH, (g + 1) * CH)
                if half == 0:
                    po = ps_o.tile([C, CH], f32, name="po", tag="o")
                    open_po[(l, g)] = po
                    for jb in range(NB // 2):
                        nc.tensor.matmul(
                            po, mn[:, jb, :], ATr[:, jb, cs],
                            start=(jb == 0), stop=False,
                            skip_group_check=True)
                else:
                    po = open_po.pop((l, g))
                    for jb in range(NB // 2, NB):
                        nc.tensor.matmul(
                            po, mn[:, jb, :], ATr[:, jb, cs],
                            start=False, stop=False,
                            skip_group_check=True)
                    nc.tensor.matmul(
                        po, wT[(l, "s")], Ht[l][:, cs],
                        start=False, stop=True, skip_group_check=True)
                    emit_evac(l, g, po)

            for g in range(IC):
                for pair in range(CH // (P * SLAB_PAIRS)):
                    emit_transpose_pair(g, pair)
                    if g > 0:
                        emit_agg_half(0, g - 1, mn1, pair)
            emit_agg_half(0, IC - 1, mn1, 0)
            emit_agg_half(0, IC - 1, mn1, 1)

            # ---- layers 2..3 ---------------------------------------------
            # Mprod for layer l+1 is interleaved right after layer l's chunk
            # g evacuates (its Ht columns are ready), smoothing the PE stream
            # across layer boundaries.
            mns = {0: mn1}
            for l in range(1, N_LAYERS):
                mns[l] = mn_pool.tile([P, NB, C], adt, name="mn", tag="mn")
            for l in range(1, N_LAYERS):
                mn = mns[l]
                if l == 1:
                    emit_mprod(l, mn)  # Ht[1] fully available by now
                for g in range(IC):
                    emit_agg_half(l, g, mn, 0)
                    emit_agg_half(l, g, mn, 1)
                    if l + 1 < N_LAYERS:
                        jb0 = g * (NB // IC)
                        emit_mprod(l + 1, mns[l + 1],
                                   range(jb0, jb0 + NB // IC))

    return nc


# ---------------------------------------------------------------------------
# Harness entry point
# ---------------------------------------------------------------------------
_NC_CACHE = {}


def _get_nc(nn_nodes):
    if nn_nodes not in _NC_CACHE:
        _NC_CACHE[nn_nodes] = build_gcn(nn_nodes)
    return _NC_CACHE[nn_nodes]


def kernel(X, A, Wm0, Ws0, b0, Wm1, Ws1, b1, Wm2, Ws2, b2, _trace=False):
    from concourse.bass_utils import run_bass_kernel_spmd

    X = np.ascontiguousarray(np.asarray(X, dtype=np.float32))
    A = np.ascontiguousarray(np.asarray(A, dtype=np.float32))
    B, NN, _C = X.shape
    assert B == 8, f"expected batch 8 (one per core), got {B}"

    shared = {
        "Wm0": np.ascontiguousarray(np.asarray(Wm0, np.float32)),
        "Ws0": np.ascontiguousarray(np.asarray(Ws0, np.float32)),
        "b0": np.ascontiguousarray(np.asarray(b0, np.float32)),
        "Wm1": np.ascontiguousarray(np.asarray(Wm1, np.float32)),
        "Ws1": np.ascontiguousarray(np.asarray(Ws1, np.float32)),
        "b1": np.ascontiguousarray(np.asarray(b1, np.float32)),
        "Wm2": np.ascontiguousarray(np.asarray(Wm2, np.float32)),
        "Ws2": np.ascontiguousarray(np.asarray(Ws2, np.float32)),
        "b2": np.ascontiguousarray(np.asarray(b2, np.float32)),
    }
    nc = _get_nc(NN)
    in_maps = [dict(shared, X=X[b], A=A[b]) for b in range(B)]
    res = run_bass_kernel_spmd(nc, in_maps, core_ids=list(range(B)),
                               trace=_trace)
    out = np.stack([res.results[b]["H"] for b in range(B)], axis=0)
    if _trace:
        return out, res
    return out
